# revision 47
# baseline (speedup 1.0000x reference)
"""Trainium2 Bass kernel for nn_Attention_66314295050336.

Sparse (threshold-pruned) multi-head attention:
    qkv  = x @ w_qkv.T          [B,N,3C]  (biases are zeros per spec)
    q,k,v heads (H=6, D=64), attn = softmax(mask(q@k.T * D**-0.5))
    mask: scores < 0.0 -> -10000 before softmax (=> weight 0 in fp32)
    out  = (attn @ v) @ w_proj.T

Sharding: pure data-parallel over batch B=8 across the 8 NeuronCores
(one batch per core, no collectives).

v2 design notes (vs the earlier Z-ones-matmul version):
  * v is stored [128, 8, 6, 65]: a ones column FIRST, then each head's
    64 v-columns.  The attn@v matmuls use M=65 so PSUM partition 0 of
    each 512-col half accumulates the softmax denominator Z for free --
    this removes all dedicated Z ones-matmuls (~49k PE rows, ~20us).
  * O is one [128,1024] fp32 PSUM tile per (pair,qc): ev head in cols
    0:512, od head in cols 512:1024, Z on partition 0, d on 1:65.
  * normalization per unit: DVE reciprocal_approx_fast directly on the
    PSUM z-row (custom DVE ops only work at partition base 0 -- hence
    the ones-first layout), ACT evacuates the O body to SBUF (frees
    the PSUM banks in ~1.3us so the next unit's attn@v can start),
    gpsimd partition_broadcast replicates 1/Z to 65 partitions, one
    TT multiply, then two DMAs place the ev/od halves on outT
    partitions 0:64 / 64:128 (engines cannot cross partitions between
    src and dst; DMA can).  The TT + outT DMAs are emitted
    deprioritized so they never head-of-line block the in-order DVE
    queue ahead of the next unit's masks.
  * ACT runs the 48 exps + the O/fin evacuations; DVE runs masks,
    qk/v casts, recips, norm TTs; gpsimd runs only the broadcasts
    (it cannot access PSUM, and bulk elementwise there is ~2.5x
    slower than DVE and head-blocks the broadcasts).
  * mask (threshold pruning): b=(e>=1) via tensor_scalar (4x mode),
    e*=b via tensor_tensor (2x mode), per single kt block (MASK_GRP=1
    keeps attn@v trailing the exp stream closely; larger batches
    serialize the pipeline and measure slower).
  * loop order: qc outer, pair inner, so proj for the first half of
    the rows overlaps the second half of attention; the last unit's
    norm runs in two q-halves so proj qt4/qt5 start early.
"""

import os
import sys

import numpy as np

for _p in ("/opt/trn_rl_repo", "/root/.axon_site/_ro/trn_rl_repo"):
    if os.path.isdir(_p) and _p not in sys.path:
        sys.path.insert(0, _p)

N = 1024
C = 384
H = 6
D = 64
SCALE = float(D) ** -0.5  # 0.125
NCORES = 8

MASK_GRP = int(os.environ.get("MASK_GRP", "1"))
TT_POOL_EVERY = int(os.environ.get("TT_POOL_EVERY", "0"))  # 0=never
USE_OU = int(os.environ.get("USE_OU", "1"))
USE_STT = int(os.environ.get("USE_STT", "0"))

_CACHE = {}


def _build():
    import concourse.bass as bass
    import concourse.mybir as mybir
    import concourse.tile as tile
    from concourse import bacc
    from contextlib import ExitStack

    F32 = mybir.dt.float32
    F16 = mybir.dt.float16
    IS_GE = mybir.AluOpType.is_ge
    EXP = mybir.ActivationFunctionType.Exp

    nc = bacc.Bacc(
        "TRN2", target_bir_lowering=False, debug=False, enable_asserts=False
    )

    xT_d = nc.dram_tensor("xT", [C, N], F16, kind="ExternalInput")
    wqkvT_d = nc.dram_tensor("wqkvT", [C, 3 * C], F16, kind="ExternalInput")
    wprojT_d = nc.dram_tensor("wprojT", [C, C], F16, kind="ExternalInput")
    out_d = nc.dram_tensor("out", [N, C], F32, kind="ExternalOutput")

    with tile.TileContext(nc) as tc, ExitStack() as ctx:
        const = ctx.enter_context(tc.tile_pool(name="const", bufs=1))
        epool = ctx.enter_context(tc.tile_pool(name="e", bufs=3))
        bpool = ctx.enter_context(tc.tile_pool(name="bn", bufs=3))
        npool = ctx.enter_context(tc.tile_pool(name="nrm", bufs=2))
        psS = ctx.enter_context(
            tc.tile_pool(name="psS", bufs=2, space=bass.MemorySpace.PSUM)
        )
        psO = ctx.enter_context(
            tc.tile_pool(name="psO", bufs=1, space=bass.MemorySpace.PSUM)
        )

        xT = const.tile([128, 3 * N], F16)  # c-tile ct -> cols [ct*N:(ct+1)*N]
        wqkv = const.tile([128, 3 * 1152], F16)  # ct -> cols [ct*1152 ...]
        wproj = const.tile([128, 3 * C], F16)
        qk = const.tile([128, 6 * N], F16)  # q pairs 0..2, k pairs 3..5
        v4 = const.tile([128, 8, 6, 65], F16)  # [nt, head, v|1]
        outT = const.tile([128, 3 * N], F16)  # pair p -> cols [p*N:(p+1)*N]

        # spread input loads over queues; xT + v-weights first (prologue
        # starts with v production)
        for ct in range(3):
            r = slice(ct * 128, (ct + 1) * 128)
            nc.sync.dma_start(xT[:, ct * N : (ct + 1) * N], xT_d[r, :])
            nc.scalar.dma_start(
                wqkv[:, ct * 1152 + 768 : (ct + 1) * 1152], wqkvT_d[r, 768:]
            )
        for ct in range(3):
            r = slice(ct * 128, (ct + 1) * 128)
            nc.scalar.dma_start(
                wqkv[:, ct * 1152 : ct * 1152 + 768], wqkvT_d[r, 0:768]
            )
            nc.sync.dma_start(wproj[:, ct * C : (ct + 1) * C], wprojT_d[r, :])
        # ones column FIRST in each head: Z lands on PSUM partition 0,
        # where the (base-0-only) gpsimd broadcast can read it directly
        nc.gpsimd.memset(v4[:, :, :, 0:1], 1.0)

        # ---------------- qkv production helpers --------------------------
        def emit_qk_half(oc, nh):
            # qkT o-chunk oc (128 rows), n-half nh -> qk cols
            ps = psS.tile([128, 512], F32, tag="f", name=f"f_qk{oc}_{nh}")
            for ct in range(3):
                nc.tensor.matmul(
                    ps[:],
                    wqkv[
                        :, ct * 1152 + oc * 128 : ct * 1152 + (oc + 1) * 128
                    ],
                    xT[:, ct * N + nh * 512 : ct * N + nh * 512 + 512],
                    start=(ct == 0),
                    stop=(ct == 2),
                )
            dst = qk[:, oc * N + nh * 512 : oc * N + nh * 512 + 512]
            nc.vector.tensor_copy(dst, ps[:])

        def emit_v_group(nt):
            ps = psS.tile([128, 384], F32, tag="f", name=f"f_v{nt}")
            for ct in range(3):
                nc.tensor.matmul(
                    ps[:],
                    xT[:, ct * N + nt * 128 : ct * N + (nt + 1) * 128],
                    wqkv[:, ct * 1152 + 768 : ct * 1152 + 1152],
                    start=(ct == 0),
                    stop=(ct == 2),
                )
            # strided copy into [6,64] segments (leaves ones cols intact)
            nc.vector.tensor_copy(v4[:, nt, :, 1:65], ps[:])

        def proj_mm(ps, qt, p3, start, stop):
            nc.tensor.matmul(
                ps[:],
                outT[:, p3 * N + qt * 128 : (p3 * N + (qt + 1) * 128)],
                wproj[:, p3 * C : (p3 + 1) * C],
                start=start,
                stop=stop,
            )

        def proj_fin(ps, qt, eng=None):
            fin = bpool.tile([128, C], F32, tag="fin", name=f"fin_{qt}")
            if eng is None:
                nc.scalar.copy(fin[:], ps[:])
            else:
                eng.tensor_copy(fin[:], ps[:])
            nc.sync.dma_start(out_d[qt * 128 : (qt + 1) * 128, :], fin[:])

        def emit_proj(qt):
            ps = psS.tile([128, C], F32, tag="f", name=f"f_pr{qt}")
            for p3 in range(3):
                proj_mm(ps, qt, p3, p3 == 0, p3 == 2)
            proj_fin(ps, qt)

        # prologue: v0-v3 + q/k pair 0
        for nt in range(4):
            emit_v_group(nt)
        for oc in (0, 3):
            for nh in range(2):
                emit_qk_half(oc, nh)

        # background queue, one item per kt slot.
        # unit order: (qc0: p0 p1 p2), (qc1: p0 p1 p2)
        bg = []
        for nt in range(4, 8):
            bg.append(lambda nt=nt: emit_v_group(nt))
        for oc in (1, 4):
            for nh in range(2):
                bg.append(lambda oc=oc, nh=nh: emit_qk_half(oc, nh))
        for oc in (2, 5):
            for nh in range(2):
                bg.append(lambda oc=oc, nh=nh: emit_qk_half(oc, nh))
        # proj for qc0 rows (qt 0..3) is appended after (p2,qc0) completes;
        # proj for qc1 (qt 4..7) runs in the tail.

        # ---------------- attention units ---------------------------------
        for qc in range(2):
            for p in range(3):
                unit = qc * 3 + p
                h_ev, h_od = 2 * p, 2 * p + 1
                qT0 = p * N
                kT0 = (3 + p) * N
                e = epool.tile([128, 8 * N], F16, tag="e", name=f"e_{unit}")
                O = psO.tile([128, 1024], F32, tag="O", name=f"O_{unit}")


                def do_av(kb):
                    st, sp = (kb == 0), (kb == 7)
                    nc.tensor.matmul(
                        O[0:65, 0:512],
                        v4[:, kb, h_ev, :],
                        e[:, kb * N : kb * N + 512],
                        start=st,
                        stop=sp,
                        skip_group_check=True,
                    )
                    nc.tensor.matmul(
                        O[0:65, 512:1024],
                        v4[:, kb, h_od, :],
                        e[:, kb * N + 512 : (kb + 1) * N],
                        start=st,
                        stop=sp,
                        skip_group_check=True,
                    )

                for kt in range(8):
                    if bg:
                        bg.pop(0)()
                    s = psS.tile(
                        [128, 1024], F32, tag="s", name=f"s_{unit}_{kt}"
                    )
                    nc.tensor.matmul(
                        s[:, 0:512],
                        qk[0:64, kT0 + kt * 128 : kT0 + (kt + 1) * 128],
                        qk[0:64, qT0 + qc * 512 : qT0 + qc * 512 + 512],
                        start=True,
                        stop=True,
                    )
                    nc.tensor.matmul(
                        s[:, 512:1024],
                        qk[64:128, kT0 + kt * 128 : kT0 + (kt + 1) * 128],
                        qk[64:128, qT0 + qc * 512 : qT0 + qc * 512 + 512],
                        start=True,
                        stop=True,
                    )
                    nc.scalar.activation(
                        e[:, kt * N : (kt + 1) * N], s[:], EXP, scale=SCALE
                    )
                    if USE_STT:
                        blk = e[:, kt * N : (kt + 1) * N]
                        nc.vector.scalar_tensor_tensor(
                            blk, s[:], 0.0, blk,
                            mybir.AluOpType.is_ge, mybir.AluOpType.mult,
                        )
                        do_av(kt)
                    elif kt % MASK_GRP == MASK_GRP - 1:
                        g0 = kt - MASK_GRP + 1
                        blk = e[:, g0 * N : (kt + 1) * N]
                        b = bpool.tile(
                            [128, MASK_GRP * N],
                            F16,
                            tag="b",
                            name=f"b_{unit}_{kt}",
                        )
                        nc.vector.tensor_scalar(b[:], blk, 1.0, None, IS_GE)
                        if TT_POOL_EVERY and (unit * (8 // MASK_GRP) + kt // MASK_GRP) % TT_POOL_EVERY == TT_POOL_EVERY - 1:
                            nc.gpsimd.tensor_mul(blk, blk, b[:])
                        else:
                            nc.vector.tensor_mul(blk, blk, b[:])
                        for kb in range(g0, kt + 1):
                            do_av(kb)

                # -------- normalization ------------------------------------
                # Z sits on PSUM partition 0 (ones col first in v).  Free
                # the O banks fast: DVE copies the z-row (lane 0) while ACT
                # evacuates the body (lanes 1:65); then broadcast + recip
                # (both base-0-only custom ops) + one fp16 2x TT, and DMA
                # places the ev/od halves on outT partitions 0:64 / 64:128.
                last = qc == 1 and p == 2
                zi = npool.tile([1, 1024], F32, tag="zi", name=f"zi_{unit}")
                oU = npool.tile([65, 1024], F16, tag="oU", name=f"oU_{unit}")
                binv = npool.tile([65, 1024], F32, tag="bi", name=f"bi_{unit}")
                ot = npool.tile([65, 1024], F16, tag="ot", name=f"ot_{unit}")
                dst0 = qT0 + qc * 512

                def norm_half(lo, hi):
                    # q-range [lo,hi) of this unit's 512 rows; ev data in
                    # cols [lo,hi), od in [512+lo, 512+hi)
                    qsl = [slice(lo, hi), slice(512 + lo, 512 + hi)]
                    for ssl in qsl:
                        nc.vector.reciprocal_approx_fast(
                            zi[0:1, ssl], O[0:1, ssl]
                        )
                        nc.scalar.copy(oU[0:65, ssl], O[0:65, ssl])
                        nc.gpsimd.partition_broadcast(
                            binv[0:65, ssl], zi[0:1, ssl]
                        )
                        nc.vector.tensor_mul(
                            ot[0:65, ssl], oU[0:65, ssl], binv[0:65, ssl]
                        )
                    nc.sync.dma_start(
                        outT[0:64, dst0 + lo : dst0 + hi], ot[1:65, qsl[0]]
                    )
                    nc.sync.dma_start(
                        outT[64:128, dst0 + lo : dst0 + hi], ot[1:65, qsl[1]]
                    )

                if last:
                    norm_half(0, 256)
                    emit_proj(4)
                    emit_proj(5)
                    norm_half(256, 512)
                else:
                    nc.vector.reciprocal_approx_fast(zi[0:1, :], O[0:1, :])
                    if USE_OU:
                        nc.scalar.copy(oU[0:65, :], O[0:65, :])
                    nc.gpsimd.partition_broadcast(binv[0:65, :], zi[0:1, :])
                    # deprioritized: the TT waits on the Pool broadcast;
                    # emitted later so it can't head-of-line block the DVE
                    # queue ahead of the next unit's masks
                    with tc.high_priority(offset=-12):
                        nc.vector.tensor_mul(
                            ot[0:65, :],
                            oU[0:65, :] if USE_OU else O[0:65, :],
                            binv[0:65, :],
                        )
                        nc.sync.dma_start(
                            outT[0:64, dst0 : dst0 + 512], ot[1:65, 0:512]
                        )
                        nc.sync.dma_start(
                            outT[64:128, dst0 : dst0 + 512],
                            ot[1:65, 512:1024],
                        )

                if qc == 0 and p == 2:
                    for qt in range(4):
                        bg.append(lambda qt=qt: emit_proj(qt))



        for qt in range(6, 8):
            emit_proj(qt)

    nc.compile()
    return nc


def get_nc():
    if "nc" not in _CACHE:
        _CACHE["nc"] = _build()
    return _CACHE["nc"]


def make_in_maps(x, w_qkv, w_proj):
    wqkvT = np.ascontiguousarray(w_qkv.T).astype(np.float16)
    wprojT = np.ascontiguousarray(w_proj.T).astype(np.float16)
    return [
        {
            "xT": np.ascontiguousarray(x[b].T).astype(np.float16),
            "wqkvT": wqkvT,
            "wprojT": wprojT,
        }
        for b in range(x.shape[0])
    ]


def kernel(x, w_qkv, b_qkv, w_proj, b_proj):
    from concourse.bass_utils import run_bass_kernel_spmd

    x = np.asarray(x)
    assert x.shape == (NCORES, N, C), x.shape
    assert not np.asarray(b_qkv).any() and not np.asarray(b_proj).any(), (
        "kernel specialized for zero biases (problem spec fill=zeros)"
    )

    nc = get_nc()
    res = run_bass_kernel_spmd(nc, make_in_maps(x, w_qkv, w_proj), list(range(NCORES)))
    out = np.stack([res.results[i]["out"] for i in range(NCORES)], axis=0)
    return out.astype(np.float32)


if __name__ == "__main__":
    nc = get_nc()
    print("built + compiled OK:", nc)


# revision 48
# speedup vs baseline: 1.0138x; 1.0138x over previous
"""Trainium2 Bass kernel for nn_Attention_66314295050336.

Sparse (threshold-pruned) multi-head attention:
    qkv  = x @ w_qkv.T          [B,N,3C]  (biases are zeros per spec)
    q,k,v heads (H=6, D=64), attn = softmax(mask(q@k.T * D**-0.5))
    mask: scores < 0.0 -> -10000 before softmax (=> weight 0 in fp32)
    out  = (attn @ v) @ w_proj.T

Sharding: pure data-parallel over batch B=8 across the 8 NeuronCores
(one batch per core, no collectives).

v2 design notes (vs the earlier Z-ones-matmul version):
  * v is stored [128, 8, 6, 65]: a ones column FIRST, then each head's
    64 v-columns.  The attn@v matmuls use M=65 so PSUM partition 0 of
    each 512-col half accumulates the softmax denominator Z for free --
    this removes all dedicated Z ones-matmuls (~49k PE rows, ~20us).
  * O is one [128,1024] fp32 PSUM tile per (pair,qc): ev head in cols
    0:512, od head in cols 512:1024, Z on partition 0, d on 1:65.
  * normalization per unit: DVE reciprocal_approx_fast directly on the
    PSUM z-row (custom DVE ops only work at partition base 0 -- hence
    the ones-first layout), ACT evacuates the O body to SBUF (frees
    the PSUM banks in ~1.3us so the next unit's attn@v can start),
    gpsimd partition_broadcast replicates 1/Z to 65 partitions, one
    TT multiply, then two DMAs place the ev/od halves on outT
    partitions 0:64 / 64:128 (engines cannot cross partitions between
    src and dst; DMA can).  The TT + outT DMAs are emitted
    deprioritized so they never head-of-line block the in-order DVE
    queue ahead of the next unit's masks.
  * ACT runs the 48 exps + the O/fin evacuations; DVE runs masks,
    qk/v casts, recips, norm TTs; gpsimd runs only the broadcasts
    (it cannot access PSUM, and bulk elementwise there is ~2.5x
    slower than DVE and head-blocks the broadcasts).
  * mask (threshold pruning): b=(e>=1) via tensor_scalar (4x mode),
    e*=b via tensor_tensor (2x mode), per single kt block (MASK_GRP=1
    keeps attn@v trailing the exp stream closely; larger batches
    serialize the pipeline and measure slower).
  * loop order: qc outer, pair inner, so proj for the first half of
    the rows overlaps the second half of attention; the last unit's
    norm runs in two q-halves so proj qt4/qt5 start early.
"""

import os
import sys

import numpy as np

for _p in ("/opt/trn_rl_repo", "/root/.axon_site/_ro/trn_rl_repo"):
    if os.path.isdir(_p) and _p not in sys.path:
        sys.path.insert(0, _p)

N = 1024
C = 384
H = 6
D = 64
SCALE = float(D) ** -0.5  # 0.125
NCORES = 8

MASK_GRP = int(os.environ.get("MASK_GRP", "1"))
TT_POOL_EVERY = int(os.environ.get("TT_POOL_EVERY", "0"))  # 0=never
USE_OU = int(os.environ.get("USE_OU", "1"))
USE_STT = int(os.environ.get("USE_STT", "0"))

_CACHE = {}


def _build():
    import concourse.bass as bass
    import concourse.mybir as mybir
    import concourse.tile as tile
    from concourse import bacc
    from contextlib import ExitStack

    F32 = mybir.dt.float32
    F16 = mybir.dt.float16
    IS_GE = mybir.AluOpType.is_ge
    EXP = mybir.ActivationFunctionType.Exp

    nc = bacc.Bacc(
        "TRN2", target_bir_lowering=False, debug=False, enable_asserts=False
    )

    xT_d = nc.dram_tensor("xT", [C, N], F16, kind="ExternalInput")
    wqkvT_d = nc.dram_tensor("wqkvT", [C, 3 * C], F16, kind="ExternalInput")
    wprojT_d = nc.dram_tensor("wprojT", [C, C], F16, kind="ExternalInput")
    out_d = nc.dram_tensor("out", [N, C], F32, kind="ExternalOutput")

    with tile.TileContext(nc) as tc, ExitStack() as ctx:
        const = ctx.enter_context(tc.tile_pool(name="const", bufs=1))
        epool = ctx.enter_context(tc.tile_pool(name="e", bufs=2))
        bpool = ctx.enter_context(tc.tile_pool(name="bn", bufs=2))
        npool = ctx.enter_context(tc.tile_pool(name="nrm", bufs=2))
        psS = ctx.enter_context(
            tc.tile_pool(name="psS", bufs=2, space=bass.MemorySpace.PSUM)
        )
        psO = ctx.enter_context(
            tc.tile_pool(name="psO", bufs=1, space=bass.MemorySpace.PSUM)
        )

        xT = const.tile([128, 3 * N], F16)  # c-tile ct -> cols [ct*N:(ct+1)*N]
        wqkv = const.tile([128, 3 * 1152], F16)  # ct -> cols [ct*1152 ...]
        wproj = const.tile([128, 3 * C], F16)
        qk = const.tile([128, 6 * N], F16)  # q pairs 0..2, k pairs 3..5
        v4 = const.tile([128, 8, 6, 65], F16)  # [nt, head, v|1]
        outT = const.tile([128, 3 * N], F16)  # pair p -> cols [p*N:(p+1)*N]

        # spread input loads over queues; xT + v-weights first (prologue
        # starts with v production)
        for ct in range(3):
            r = slice(ct * 128, (ct + 1) * 128)
            nc.sync.dma_start(xT[:, ct * N : (ct + 1) * N], xT_d[r, :])
            nc.scalar.dma_start(
                wqkv[:, ct * 1152 + 768 : (ct + 1) * 1152], wqkvT_d[r, 768:]
            )
        for ct in range(3):
            r = slice(ct * 128, (ct + 1) * 128)
            nc.scalar.dma_start(
                wqkv[:, ct * 1152 : ct * 1152 + 768], wqkvT_d[r, 0:768]
            )
            nc.sync.dma_start(wproj[:, ct * C : (ct + 1) * C], wprojT_d[r, :])
        # ones column FIRST in each head: Z lands on PSUM partition 0,
        # where the (base-0-only) gpsimd broadcast can read it directly
        nc.gpsimd.memset(v4[:, :, :, 0:1], 1.0)

        # ---------------- qkv production helpers --------------------------
        def emit_qk_half(oc, nh):
            # qkT o-chunk oc (128 rows), n-half nh -> qk cols
            ps = psS.tile([128, 512], F32, tag="f", name=f"f_qk{oc}_{nh}")
            for ct in range(3):
                nc.tensor.matmul(
                    ps[:],
                    wqkv[
                        :, ct * 1152 + oc * 128 : ct * 1152 + (oc + 1) * 128
                    ],
                    xT[:, ct * N + nh * 512 : ct * N + nh * 512 + 512],
                    start=(ct == 0),
                    stop=(ct == 2),
                )
            dst = qk[:, oc * N + nh * 512 : oc * N + nh * 512 + 512]
            nc.vector.tensor_copy(dst, ps[:])

        def emit_v_group(nt):
            ps = psS.tile([128, 384], F32, tag="f", name=f"f_v{nt}")
            for ct in range(3):
                nc.tensor.matmul(
                    ps[:],
                    xT[:, ct * N + nt * 128 : ct * N + (nt + 1) * 128],
                    wqkv[:, ct * 1152 + 768 : ct * 1152 + 1152],
                    start=(ct == 0),
                    stop=(ct == 2),
                )
            # strided copy into [6,64] segments (leaves ones cols intact)
            nc.vector.tensor_copy(v4[:, nt, :, 1:65], ps[:])

        def proj_mm(ps, qt, p3, start, stop):
            nc.tensor.matmul(
                ps[:],
                outT[:, p3 * N + qt * 128 : (p3 * N + (qt + 1) * 128)],
                wproj[:, p3 * C : (p3 + 1) * C],
                start=start,
                stop=stop,
            )

        def proj_fin(ps, qt, eng=None):
            fin = bpool.tile([128, C], F32, tag="fin", name=f"fin_{qt}")
            if eng is None:
                nc.scalar.copy(fin[:], ps[:])
            else:
                eng.tensor_copy(fin[:], ps[:])
            nc.sync.dma_start(out_d[qt * 128 : (qt + 1) * 128, :], fin[:])

        def emit_proj(qt):
            ps = psS.tile([128, C], F32, tag="f", name=f"f_pr{qt}")
            for p3 in range(3):
                proj_mm(ps, qt, p3, p3 == 0, p3 == 2)
            proj_fin(ps, qt)

        # prologue: v0-v3 + q/k pair 0
        for nt in range(4):
            emit_v_group(nt)
        for oc in (0, 3):
            for nh in range(2):
                emit_qk_half(oc, nh)

        # background queue, one item per kt slot.
        # unit order: (qc0: p0 p1 p2), (qc1: p0 p1 p2)
        bg = []
        for nt in range(4, 8):
            bg.append(lambda nt=nt: emit_v_group(nt))
        for oc in (1, 4):
            for nh in range(2):
                bg.append(lambda oc=oc, nh=nh: emit_qk_half(oc, nh))
        for oc in (2, 5):
            for nh in range(2):
                bg.append(lambda oc=oc, nh=nh: emit_qk_half(oc, nh))
        # proj for qc0 rows (qt 0..3) is appended after (p2,qc0) completes;
        # proj for qc1 (qt 4..7) runs in the tail.

        # ---------------- attention units ---------------------------------
        for qc in range(2):
            for p in range(3):
                unit = qc * 3 + p
                h_ev, h_od = 2 * p, 2 * p + 1
                qT0 = p * N
                kT0 = (3 + p) * N
                e = epool.tile([128, 8 * N], F16, tag="e", name=f"e_{unit}")
                O = psO.tile([128, 1024], F32, tag="O", name=f"O_{unit}")


                def do_av(kb):
                    st, sp = (kb == 0), (kb == 7)
                    nc.tensor.matmul(
                        O[0:65, 0:512],
                        v4[:, kb, h_ev, :],
                        e[:, kb * N : kb * N + 512],
                        start=st,
                        stop=sp,
                        skip_group_check=True,
                    )
                    nc.tensor.matmul(
                        O[0:65, 512:1024],
                        v4[:, kb, h_od, :],
                        e[:, kb * N + 512 : (kb + 1) * N],
                        start=st,
                        stop=sp,
                        skip_group_check=True,
                    )

                for kt in range(8):
                    if bg:
                        bg.pop(0)()
                    s = psS.tile(
                        [128, 1024], F32, tag="s", name=f"s_{unit}_{kt}"
                    )
                    nc.tensor.matmul(
                        s[:, 0:512],
                        qk[0:64, kT0 + kt * 128 : kT0 + (kt + 1) * 128],
                        qk[0:64, qT0 + qc * 512 : qT0 + qc * 512 + 512],
                        start=True,
                        stop=True,
                    )
                    nc.tensor.matmul(
                        s[:, 512:1024],
                        qk[64:128, kT0 + kt * 128 : kT0 + (kt + 1) * 128],
                        qk[64:128, qT0 + qc * 512 : qT0 + qc * 512 + 512],
                        start=True,
                        stop=True,
                    )
                    nc.scalar.activation(
                        e[:, kt * N : (kt + 1) * N], s[:], EXP, scale=SCALE
                    )
                    if USE_STT:
                        blk = e[:, kt * N : (kt + 1) * N]
                        nc.vector.scalar_tensor_tensor(
                            blk, s[:], 0.0, blk,
                            mybir.AluOpType.is_ge, mybir.AluOpType.mult,
                        )
                        do_av(kt)
                    elif kt % MASK_GRP == MASK_GRP - 1:
                        g0 = kt - MASK_GRP + 1
                        blk = e[:, g0 * N : (kt + 1) * N]
                        b = bpool.tile(
                            [128, MASK_GRP * N],
                            F16,
                            tag="b",
                            name=f"b_{unit}_{kt}",
                        )
                        nc.vector.tensor_scalar(b[:], blk, 1.0, None, IS_GE)
                        if TT_POOL_EVERY and (unit * (8 // MASK_GRP) + kt // MASK_GRP) % TT_POOL_EVERY == TT_POOL_EVERY - 1:
                            nc.gpsimd.tensor_mul(blk, blk, b[:])
                        else:
                            nc.vector.tensor_mul(blk, blk, b[:])
                        for kb in range(g0, kt + 1):
                            do_av(kb)

                # -------- normalization ------------------------------------
                # Z sits on PSUM partition 0 (ones col first in v).  Free
                # the O banks fast: DVE copies the z-row (lane 0) while ACT
                # evacuates the body (lanes 1:65); then broadcast + recip
                # (both base-0-only custom ops) + one fp16 2x TT, and DMA
                # places the ev/od halves on outT partitions 0:64 / 64:128.
                last = qc == 1 and p == 2
                zi = npool.tile([1, 1024], F32, tag="zi", name=f"zi_{unit}")
                oU = npool.tile([65, 1024], F16, tag="oU", name=f"oU_{unit}")
                binv = npool.tile([65, 1024], F32, tag="bi", name=f"bi_{unit}")
                ot = npool.tile([65, 1024], F16, tag="ot", name=f"ot_{unit}")
                dst0 = qT0 + qc * 512

                def norm_half(lo, hi):
                    # q-range [lo,hi) of this unit's 512 rows; ev data in
                    # cols [lo,hi), od in [512+lo, 512+hi)
                    qsl = [slice(lo, hi), slice(512 + lo, 512 + hi)]
                    for ssl in qsl:
                        nc.vector.reciprocal_approx_fast(
                            zi[0:1, ssl], O[0:1, ssl]
                        )
                        nc.scalar.copy(oU[0:65, ssl], O[0:65, ssl])
                        nc.gpsimd.partition_broadcast(
                            binv[0:65, ssl], zi[0:1, ssl]
                        )
                        nc.vector.tensor_mul(
                            ot[0:65, ssl], oU[0:65, ssl], binv[0:65, ssl]
                        )
                    nc.sync.dma_start(
                        outT[0:64, dst0 + lo : dst0 + hi], ot[1:65, qsl[0]]
                    )
                    nc.sync.dma_start(
                        outT[64:128, dst0 + lo : dst0 + hi], ot[1:65, qsl[1]]
                    )

                if last:
                    norm_half(0, 256)
                    emit_proj(4)
                    emit_proj(5)
                    norm_half(256, 512)
                else:
                    nc.vector.reciprocal_approx_fast(zi[0:1, :], O[0:1, :])
                    if USE_OU:
                        nc.scalar.copy(oU[0:65, :], O[0:65, :])
                    nc.gpsimd.partition_broadcast(binv[0:65, :], zi[0:1, :])
                    # deprioritized: the TT waits on the Pool broadcast;
                    # emitted later so it can't head-of-line block the DVE
                    # queue ahead of the next unit's masks
                    with tc.high_priority(offset=-12):
                        nc.vector.tensor_mul(
                            ot[0:65, :],
                            oU[0:65, :] if USE_OU else O[0:65, :],
                            binv[0:65, :],
                        )
                        nc.sync.dma_start(
                            outT[0:64, dst0 : dst0 + 512], ot[1:65, 0:512]
                        )
                        nc.sync.dma_start(
                            outT[64:128, dst0 : dst0 + 512],
                            ot[1:65, 512:1024],
                        )

                if qc == 0 and p == 2:
                    for qt in range(4):
                        bg.append(lambda qt=qt: emit_proj(qt))



        for qt in range(6, 8):
            emit_proj(qt)

    nc.compile()
    return nc


def get_nc():
    if "nc" not in _CACHE:
        _CACHE["nc"] = _build()
    return _CACHE["nc"]


def make_in_maps(x, w_qkv, w_proj):
    wqkvT = np.ascontiguousarray(w_qkv.T).astype(np.float16)
    wprojT = np.ascontiguousarray(w_proj.T).astype(np.float16)
    return [
        {
            "xT": np.ascontiguousarray(x[b].T).astype(np.float16),
            "wqkvT": wqkvT,
            "wprojT": wprojT,
        }
        for b in range(x.shape[0])
    ]


def kernel(x, w_qkv, b_qkv, w_proj, b_proj):
    from concourse.bass_utils import run_bass_kernel_spmd

    x = np.asarray(x)
    assert x.shape == (NCORES, N, C), x.shape
    assert not np.asarray(b_qkv).any() and not np.asarray(b_proj).any(), (
        "kernel specialized for zero biases (problem spec fill=zeros)"
    )

    nc = get_nc()
    res = run_bass_kernel_spmd(nc, make_in_maps(x, w_qkv, w_proj), list(range(NCORES)))
    out = np.stack([res.results[i]["out"] for i in range(NCORES)], axis=0)
    return out.astype(np.float32)


if __name__ == "__main__":
    nc = get_nc()
    print("built + compiled OK:", nc)


# revision 49
# speedup vs baseline: 1.0312x; 1.0171x over previous
"""Trainium2 Bass kernel for nn_Attention_66314295050336.

Sparse (threshold-pruned) multi-head attention:
    qkv  = x @ w_qkv.T          [B,N,3C]  (biases are zeros per spec)
    q,k,v heads (H=6, D=64), attn = softmax(mask(q@k.T * D**-0.5))
    mask: scores < 0.0 -> -10000 before softmax (=> weight 0 in fp32)
    out  = (attn @ v) @ w_proj.T

Sharding: pure data-parallel over batch B=8 across the 8 NeuronCores
(one batch per core, no collectives).

v2 design notes (vs the earlier Z-ones-matmul version):
  * v is stored [128, 8, 6, 65]: a ones column FIRST, then each head's
    64 v-columns.  The attn@v matmuls use M=65 so PSUM partition 0 of
    each 512-col half accumulates the softmax denominator Z for free --
    this removes all dedicated Z ones-matmuls (~49k PE rows, ~20us).
  * O is one [128,1024] fp32 PSUM tile per (pair,qc): ev head in cols
    0:512, od head in cols 512:1024, Z on partition 0, d on 1:65.
  * normalization per unit: DVE reciprocal_approx_fast directly on the
    PSUM z-row (custom DVE ops only work at partition base 0 -- hence
    the ones-first layout), ACT evacuates the O body to SBUF (frees
    the PSUM banks in ~1.3us so the next unit's attn@v can start),
    gpsimd partition_broadcast replicates 1/Z to 65 partitions, one
    TT multiply, then two DMAs place the ev/od halves on outT
    partitions 0:64 / 64:128 (engines cannot cross partitions between
    src and dst; DMA can).  The TT + outT DMAs are emitted
    deprioritized so they never head-of-line block the in-order DVE
    queue ahead of the next unit's masks.
  * ACT runs the 48 exps + the O/fin evacuations; DVE runs masks,
    qk/v casts, recips, norm TTs; gpsimd runs only the broadcasts
    (it cannot access PSUM, and bulk elementwise there is ~2.5x
    slower than DVE and head-blocks the broadcasts).
  * mask (threshold pruning): b=(e>=1) via tensor_scalar (4x mode),
    e*=b via tensor_tensor (2x mode), per single kt block (MASK_GRP=1
    keeps attn@v trailing the exp stream closely; larger batches
    serialize the pipeline and measure slower).
  * loop order: qc outer, pair inner, so proj for the first half of
    the rows overlaps the second half of attention; the last unit's
    norm runs in two q-halves so proj qt4/qt5 start early.
"""

import os
import sys

import numpy as np

for _p in ("/opt/trn_rl_repo", "/root/.axon_site/_ro/trn_rl_repo"):
    if os.path.isdir(_p) and _p not in sys.path:
        sys.path.insert(0, _p)

N = 1024
C = 384
H = 6
D = 64
SCALE = float(D) ** -0.5  # 0.125
NCORES = 8

MASK_GRP = int(os.environ.get("MASK_GRP", "1"))
TT_POOL_EVERY = int(os.environ.get("TT_POOL_EVERY", "0"))  # 0=never
USE_OU = int(os.environ.get("USE_OU", "1"))
USE_STT = int(os.environ.get("USE_STT", "0"))

_CACHE = {}


def _build():
    import concourse.bass as bass
    import concourse.mybir as mybir
    import concourse.tile as tile
    from concourse import bacc
    from contextlib import ExitStack

    F32 = mybir.dt.float32
    F16 = mybir.dt.float16
    IS_GE = mybir.AluOpType.is_ge
    EXP = mybir.ActivationFunctionType.Exp

    nc = bacc.Bacc(
        "TRN2", target_bir_lowering=False, debug=False, enable_asserts=False
    )

    xT_d = nc.dram_tensor("xT", [C, N], F16, kind="ExternalInput")
    wqkvT_d = nc.dram_tensor("wqkvT", [C, 3 * C], F16, kind="ExternalInput")
    wprojT_d = nc.dram_tensor("wprojT", [C, C], F16, kind="ExternalInput")
    out_d = nc.dram_tensor("out", [N, C], F32, kind="ExternalOutput")

    with tile.TileContext(nc) as tc, ExitStack() as ctx:
        const = ctx.enter_context(tc.tile_pool(name="const", bufs=1))
        epool = ctx.enter_context(tc.tile_pool(name="e", bufs=2))
        bpool = ctx.enter_context(tc.tile_pool(name="bn", bufs=2))
        npool = ctx.enter_context(tc.tile_pool(name="nrm", bufs=2))
        psS = ctx.enter_context(
            tc.tile_pool(name="psS", bufs=2, space=bass.MemorySpace.PSUM)
        )
        psO = ctx.enter_context(
            tc.tile_pool(name="psO", bufs=1, space=bass.MemorySpace.PSUM)
        )

        xT = const.tile([128, 3 * N], F16)  # c-tile ct -> cols [ct*N:(ct+1)*N]
        wqkv = const.tile([128, 3 * 1152], F16)  # ct -> cols [ct*1152 ...]
        wproj = const.tile([128, 3 * C], F16)
        qk = const.tile([128, 6 * N], F16)  # q pairs 0..2, k pairs 3..5
        v4 = const.tile([128, 8, 6, 65], F16)  # [nt, head, v|1]
        outT = const.tile([128, 3 * N], F16)  # pair p -> cols [p*N:(p+1)*N]

        # spread input loads over queues; xT + v-weights first (prologue
        # starts with v production)
        for ct in range(3):
            r = slice(ct * 128, (ct + 1) * 128)
            nc.sync.dma_start(xT[:, ct * N : (ct + 1) * N], xT_d[r, :])
            nc.scalar.dma_start(
                wqkv[:, ct * 1152 + 768 : (ct + 1) * 1152], wqkvT_d[r, 768:]
            )
        for ct in range(3):
            r = slice(ct * 128, (ct + 1) * 128)
            nc.scalar.dma_start(
                wqkv[:, ct * 1152 : ct * 1152 + 768], wqkvT_d[r, 0:768]
            )
            nc.sync.dma_start(wproj[:, ct * C : (ct + 1) * C], wprojT_d[r, :])
        # ones column FIRST in each head: Z lands on PSUM partition 0,
        # where the (base-0-only) gpsimd broadcast can read it directly
        nc.gpsimd.memset(v4[:, :, :, 0:1], 1.0)

        # ---------------- qkv production helpers --------------------------
        def emit_qk_half(oc, nh):
            # qkT o-chunk oc (128 rows), n-half nh -> qk cols
            ps = psS.tile([128, 512], F32, tag="f", name=f"f_qk{oc}_{nh}")
            for ct in range(3):
                nc.tensor.matmul(
                    ps[:],
                    wqkv[
                        :, ct * 1152 + oc * 128 : ct * 1152 + (oc + 1) * 128
                    ],
                    xT[:, ct * N + nh * 512 : ct * N + nh * 512 + 512],
                    start=(ct == 0),
                    stop=(ct == 2),
                )
            dst = qk[:, oc * N + nh * 512 : oc * N + nh * 512 + 512]
            nc.vector.tensor_copy(dst, ps[:])

        def emit_v_group(nt):
            ps = psS.tile([128, 384], F32, tag="f", name=f"f_v{nt}")
            for ct in range(3):
                nc.tensor.matmul(
                    ps[:],
                    xT[:, ct * N + nt * 128 : ct * N + (nt + 1) * 128],
                    wqkv[:, ct * 1152 + 768 : ct * 1152 + 1152],
                    start=(ct == 0),
                    stop=(ct == 2),
                )
            # strided copy into [6,64] segments (leaves ones cols intact).
            # On ACT: prologue copies run while ACT is idle; bg ones are
            # deprioritized so they never head-block the exp stream.
            if nt < 4:
                nc.scalar.copy(v4[:, nt, :, 1:65], ps[:])
            else:
                with tc.high_priority(offset=-16):
                    nc.scalar.copy(v4[:, nt, :, 1:65], ps[:])

        def proj_mm(ps, qt, p3, start, stop):
            nc.tensor.matmul(
                ps[:],
                outT[:, p3 * N + qt * 128 : (p3 * N + (qt + 1) * 128)],
                wproj[:, p3 * C : (p3 + 1) * C],
                start=start,
                stop=stop,
            )

        def proj_fin(ps, qt, eng=None):
            fin = bpool.tile([128, C], F32, tag="fin", name=f"fin_{qt}")
            if eng is None:
                nc.scalar.copy(fin[:], ps[:])
            else:
                eng.tensor_copy(fin[:], ps[:])
            nc.sync.dma_start(out_d[qt * 128 : (qt + 1) * 128, :], fin[:])

        def emit_proj(qt):
            ps = psS.tile([128, C], F32, tag="f", name=f"f_pr{qt}")
            for p3 in range(3):
                proj_mm(ps, qt, p3, p3 == 0, p3 == 2)
            proj_fin(ps, qt)

        # prologue: v0-v3 + q/k pair 0
        for nt in range(4):
            emit_v_group(nt)
        for oc in (0, 3):
            for nh in range(2):
                emit_qk_half(oc, nh)

        # background queue, one item per kt slot.
        # unit order: (qc0: p0 p1 p2), (qc1: p0 p1 p2)
        bg = []
        for nt in range(4, 8):
            bg.append(lambda nt=nt: emit_v_group(nt))
        for oc in (1, 4):
            for nh in range(2):
                bg.append(lambda oc=oc, nh=nh: emit_qk_half(oc, nh))
        for oc in (2, 5):
            for nh in range(2):
                bg.append(lambda oc=oc, nh=nh: emit_qk_half(oc, nh))
        # proj for qc0 rows (qt 0..3) is appended after (p2,qc0) completes;
        # proj for qc1 (qt 4..7) runs in the tail.

        # ---------------- attention units ---------------------------------
        for qc in range(2):
            for p in range(3):
                unit = qc * 3 + p
                h_ev, h_od = 2 * p, 2 * p + 1
                qT0 = p * N
                kT0 = (3 + p) * N
                e = epool.tile([128, 8 * N], F16, tag="e", name=f"e_{unit}")
                O = psO.tile([128, 1024], F32, tag="O", name=f"O_{unit}")


                def do_av(kb):
                    st, sp = (kb == 0), (kb == 7)
                    nc.tensor.matmul(
                        O[0:65, 0:512],
                        v4[:, kb, h_ev, :],
                        e[:, kb * N : kb * N + 512],
                        start=st,
                        stop=sp,
                        skip_group_check=True,
                    )
                    nc.tensor.matmul(
                        O[0:65, 512:1024],
                        v4[:, kb, h_od, :],
                        e[:, kb * N + 512 : (kb + 1) * N],
                        start=st,
                        stop=sp,
                        skip_group_check=True,
                    )

                for kt in range(8):
                    if bg:
                        bg.pop(0)()
                    s = psS.tile(
                        [128, 1024], F32, tag="s", name=f"s_{unit}_{kt}"
                    )
                    nc.tensor.matmul(
                        s[:, 0:512],
                        qk[0:64, kT0 + kt * 128 : kT0 + (kt + 1) * 128],
                        qk[0:64, qT0 + qc * 512 : qT0 + qc * 512 + 512],
                        start=True,
                        stop=True,
                    )
                    nc.tensor.matmul(
                        s[:, 512:1024],
                        qk[64:128, kT0 + kt * 128 : kT0 + (kt + 1) * 128],
                        qk[64:128, qT0 + qc * 512 : qT0 + qc * 512 + 512],
                        start=True,
                        stop=True,
                    )
                    nc.scalar.activation(
                        e[:, kt * N : (kt + 1) * N], s[:], EXP, scale=SCALE
                    )
                    if USE_STT:
                        blk = e[:, kt * N : (kt + 1) * N]
                        nc.vector.scalar_tensor_tensor(
                            blk, s[:], 0.0, blk,
                            mybir.AluOpType.is_ge, mybir.AluOpType.mult,
                        )
                        do_av(kt)
                    elif kt % MASK_GRP == MASK_GRP - 1:
                        g0 = kt - MASK_GRP + 1
                        blk = e[:, g0 * N : (kt + 1) * N]
                        b = bpool.tile(
                            [128, MASK_GRP * N],
                            F16,
                            tag="b",
                            name=f"b_{unit}_{kt}",
                        )
                        nc.vector.tensor_scalar(b[:], blk, 1.0, None, IS_GE)
                        if TT_POOL_EVERY and (unit * (8 // MASK_GRP) + kt // MASK_GRP) % TT_POOL_EVERY == TT_POOL_EVERY - 1:
                            nc.gpsimd.tensor_mul(blk, blk, b[:])
                        else:
                            nc.vector.tensor_mul(blk, blk, b[:])
                        for kb in range(g0, kt + 1):
                            do_av(kb)

                # -------- normalization ------------------------------------
                # Z sits on PSUM partition 0 (ones col first in v).  Free
                # the O banks fast: DVE copies the z-row (lane 0) while ACT
                # evacuates the body (lanes 1:65); then broadcast + recip
                # (both base-0-only custom ops) + one fp16 2x TT, and DMA
                # places the ev/od halves on outT partitions 0:64 / 64:128.
                last = qc == 1 and p == 2
                zi = npool.tile([1, 1024], F32, tag="zi", name=f"zi_{unit}")
                oU = npool.tile([65, 1024], F16, tag="oU", name=f"oU_{unit}")
                binv = npool.tile([65, 1024], F32, tag="bi", name=f"bi_{unit}")
                ot = npool.tile([65, 1024], F16, tag="ot", name=f"ot_{unit}")
                dst0 = qT0 + qc * 512

                def norm_half(lo, hi):
                    # q-range [lo,hi) of this unit's 512 rows; ev data in
                    # cols [lo,hi), od in [512+lo, 512+hi)
                    qsl = [slice(lo, hi), slice(512 + lo, 512 + hi)]
                    for ssl in qsl:
                        nc.vector.reciprocal_approx_fast(
                            zi[0:1, ssl], O[0:1, ssl]
                        )
                        nc.scalar.copy(oU[0:65, ssl], O[0:65, ssl])
                        nc.gpsimd.partition_broadcast(
                            binv[0:65, ssl], zi[0:1, ssl]
                        )
                        nc.vector.tensor_mul(
                            ot[0:65, ssl], oU[0:65, ssl], binv[0:65, ssl]
                        )
                    nc.sync.dma_start(
                        outT[0:64, dst0 + lo : dst0 + hi], ot[1:65, qsl[0]]
                    )
                    nc.sync.dma_start(
                        outT[64:128, dst0 + lo : dst0 + hi], ot[1:65, qsl[1]]
                    )

                if last:
                    norm_half(0, 256)
                    emit_proj(4)
                    emit_proj(5)
                    norm_half(256, 512)
                else:
                    nc.vector.reciprocal_approx_fast(zi[0:1, :], O[0:1, :])
                    if USE_OU:
                        nc.scalar.copy(oU[0:65, :], O[0:65, :])
                    nc.gpsimd.partition_broadcast(binv[0:65, :], zi[0:1, :])
                    # deprioritized: the TT waits on the Pool broadcast;
                    # emitted later so it can't head-of-line block the DVE
                    # queue ahead of the next unit's masks
                    with tc.high_priority(offset=-12):
                        nc.vector.tensor_mul(
                            ot[0:65, :],
                            oU[0:65, :] if USE_OU else O[0:65, :],
                            binv[0:65, :],
                        )
                        nc.sync.dma_start(
                            outT[0:64, dst0 : dst0 + 512], ot[1:65, 0:512]
                        )
                        nc.sync.dma_start(
                            outT[64:128, dst0 : dst0 + 512],
                            ot[1:65, 512:1024],
                        )

                if qc == 0 and p == 2:
                    for qt in range(4):
                        bg.append(lambda qt=qt: emit_proj(qt))



        for qt in range(6, 8):
            emit_proj(qt)

    nc.compile()
    return nc


def get_nc():
    if "nc" not in _CACHE:
        _CACHE["nc"] = _build()
    return _CACHE["nc"]


def make_in_maps(x, w_qkv, w_proj):
    wqkvT = np.ascontiguousarray(w_qkv.T).astype(np.float16)
    wprojT = np.ascontiguousarray(w_proj.T).astype(np.float16)
    return [
        {
            "xT": np.ascontiguousarray(x[b].T).astype(np.float16),
            "wqkvT": wqkvT,
            "wprojT": wprojT,
        }
        for b in range(x.shape[0])
    ]


def kernel(x, w_qkv, b_qkv, w_proj, b_proj):
    from concourse.bass_utils import run_bass_kernel_spmd

    x = np.asarray(x)
    assert x.shape == (NCORES, N, C), x.shape
    assert not np.asarray(b_qkv).any() and not np.asarray(b_proj).any(), (
        "kernel specialized for zero biases (problem spec fill=zeros)"
    )

    nc = get_nc()
    res = run_bass_kernel_spmd(nc, make_in_maps(x, w_qkv, w_proj), list(range(NCORES)))
    out = np.stack([res.results[i]["out"] for i in range(NCORES)], axis=0)
    return out.astype(np.float32)


if __name__ == "__main__":
    nc = get_nc()
    print("built + compiled OK:", nc)
